# revision 31
# baseline (speedup 1.0000x reference)
"""Trainium2 Bass kernel for nn_DecoderCBatchNorm_63788854280467.

Decoder with bilinear plane-feature interpolation + small residual MLP.
Data-parallel over batch: 16 batches -> 8 NeuronCores (2 each).

v2 layout strategy:
- Supercell table rows are (d, corner)-interleaved fp16, so the gather lands
  features in a layout where the bilinear weight multiply is a fully packed
  16-bit DVE op (2x rate) and the 16 (view, corner) terms per (point, d) are
  contiguous for a packed fp16 add-tree.
- Projection/clamp chain uses host-folded coefficients (C/(0.55*den*interval))
  so xy comes from 6 wide vector ops instead of exact-division ladders.
- MLP matmuls, residual adds (via identity-matmul PSUM accumulation), and
  transposes all run 16-bit on the PE; relu/copies ride the scalar engine.
"""

import sys

sys.path.insert(0, "/opt/trn_rl_repo")

import numpy as np

import concourse.bass as bass
import concourse.bacc as bacc
import concourse.mybir as mybir
from concourse import tile, library_config
from concourse.bass_utils import run_bass_kernel_spmd
from concourse.masks import make_identity

F32 = mybir.dt.float32
FP16 = mybir.dt.float16
I16 = mybir.dt.int16
AOT = mybir.AluOpType
AFT = mybir.ActivationFunctionType

B, T, L, H, W, D = 16, 4096, 4, 128, 128, 32
MAX_DIM = 0.55
NCORES = 8
BPC = B // NCORES          # batches per core = 2
MAGIC = 12582912.0         # 1.5 * 2^23 : f32 RNE rounding constant
INTERVAL = np.float32(2.0) / np.float32(H - 1)   # f32(2/127), matches jnp

# cst column map (per-batch blocks of 8 = (l, coord) pairs)
_CN_C0 = 0        # 16 cols: coef for p0, idx b*8 + 2l+c
_CN_C1 = 16
_CN_C2 = 32
_CN_CC = 48       # 16 cols: additive const (= 1/interval)
_CN_B0 = 64       # 5 cols
_CN_B1 = 69       # 5 cols
_CN_FOB = 74      # 1 col
_CN = 75

_cache = {}


def _ap3(tile_ap, dims, offset_elems):
    """Build an AP with explicit free dims [(step, count), ...] on a tile AP."""
    base = tile_ap
    ap = [list(base.ap[0])] + [[s, c] for (s, c) in dims]
    return bass.AP(base.tensor, base.offset + offset_elems, ap)


def _build_nc(iters=0, unroll=1):
    """Build the per-core program. iters>0 wraps the body in a timing loop."""
    nc = bacc.Bacc("TRN2", target_bir_lowering=False, debug=False)

    ct = nc.dram_tensor("ct", [BPC * L, H * W, 4 * D], FP16, kind="ExternalInput")
    p_d = nc.dram_tensor("p", [BPC, T, 3], F32, kind="ExternalInput")
    pt16_d = nc.dram_tensor("pt16", [BPC, 16, T // 4], FP16, kind="ExternalInput")
    cst_d = nc.dram_tensor("cst", [128, _CN], F32, kind="ExternalInput")
    wbd_d = nc.dram_tensor("wbd", [128, 1280], FP16, kind="ExternalInput")
    emat_d = nc.dram_tensor("emat", [128, 1024], F32, kind="ExternalInput")
    wrep_d = nc.dram_tensor("wrep", [16, 128], FP16, kind="ExternalInput")
    fob_d = nc.dram_tensor("fob", [128, 4], FP16, kind="ExternalInput")
    o_d = nc.dram_tensor("o", [BPC, T], F32, kind="ExternalOutput")

    with tile.TileContext(nc) as tc:
        nc.gpsimd.load_library(library_config.mlp)
        with tc.tile_pool(name="sb", bufs=2) as pl, \
             tc.tile_pool(name="cs", bufs=1) as cs, \
             tc.tile_pool(name="ps", bufs=1, space="PSUM") as ps:

            ident = cs.tile([128, 128], F32)
            make_identity(nc, ident[:])
            identh = cs.tile([128, 128], FP16)
            nc.vector.tensor_copy(out=identh[:], in_=ident[:])
            cst = cs.tile([128, _CN], F32)
            nc.sync.dma_start(out=cst[:], in_=cst_d.ap())
            wbd = cs.tile([128, 1280], FP16)
            nc.sync.dma_start(out=wbd[:], in_=wbd_d.ap())
            emat = cs.tile([128, 1024], F32)
            nc.sync.dma_start(out=emat[:], in_=emat_d.ap())
            wrep = cs.tile([16, 128], FP16)
            nc.sync.dma_start(out=wrep[:], in_=wrep_d.ap())
            fob = cs.tile([128, 4], FP16)
            nc.sync.dma_start(out=fob[:], in_=fob_d.ap())

            NU = 2 * BPC      # pipeline units: (b, th)
            UNITS = [(b, th) for b in range(BPC) for th in range(2)]

            def body():
                st = {u: {} for u in range(NU)}   # per-unit tiles
                bst = {}                          # per-batch tiles

                def tt(o, a, bb, op):
                    nc.vector.tensor_tensor(out=o, in0=a, in1=bb, op=op)

                # ---- stage A (per batch): loads + xy/round chain -----------
                for b in range(BPC):
                    p_sb = pl.tile([128, 96], F32, tag="p", name="p_sb")
                    nc.sync.dma_start(
                        out=p_sb[:],
                        in_=p_d.ap()[b].rearrange("(q a) j -> q (a j)", a=32))
                    pt16 = pl.tile([16, T // 4], FP16, tag="pt16", name="pt16")
                    nc.sync.dma_start(out=pt16[:], in_=pt16_d.ap()[b])

                    def cc(base):   # per-column const broadcast over m
                        return _ap3(cst[:], [(0, 32), (1, 8)], base + 8 * b)

                    def pj(j):      # p coord j broadcast over the 8 lc cols
                        return _ap3(p_sb[:], [(3, 32), (0, 8)], j)

                    def t256(tag):
                        return pl.tile([128, 256], F32, tag=tag, name=tag)

                    # xy = sum_j p_j * coef_j + const  [128, 8m+lc]
                    M1t = t256("xm1")
                    tt(M1t[:], pj(0), cc(_CN_C0), AOT.mult)
                    M2t = t256("xm2")
                    tt(M2t[:], pj(1), cc(_CN_C1), AOT.mult)
                    M12 = t256("xm12")
                    tt(M12[:], M1t[:], M2t[:], AOT.add)
                    M3t = t256("xm1")
                    tt(M3t[:], pj(2), cc(_CN_C2), AOT.mult)
                    M3c = t256("xm2")
                    tt(M3c[:], M3t[:], cc(_CN_CC), AOT.add)
                    XYC = t256("xm1")
                    tt(XYC[:], M12[:], M3c[:], AOT.add)

                    # clamp + where(>=127 -> 126.9)
                    XY1 = t256("xm2")
                    nc.vector.tensor_scalar(out=XY1[:], in0=XYC[:], scalar1=0.0,
                                            scalar2=200.0, op0=AOT.max, op1=AOT.min)
                    Msk = t256("xm12")
                    nc.vector.tensor_scalar(out=Msk[:], in0=XY1[:], scalar1=127.0,
                                            scalar2=None, op0=AOT.is_ge)
                    Dd = t256("xm1")
                    nc.vector.tensor_scalar(out=Dd[:], in0=XY1[:], scalar1=126.9,
                                            scalar2=None, op0=AOT.subtract)
                    MD = t256("xm3")
                    tt(MD[:], Msk[:], Dd[:], AOT.mult)
                    XY2 = t256("xy2")
                    tt(XY2[:], XY1[:], MD[:], AOT.subtract)

                    # round L/U (RNE), D2 = 1-dx, SEL
                    L1 = t256("xm1")
                    nc.vector.tensor_scalar(out=L1[:], in0=XY2[:], scalar1=-0.5,
                                            scalar2=MAGIC, op0=AOT.add, op1=AOT.add)
                    Lt = t256("lt")
                    nc.vector.tensor_scalar(out=Lt[:], in0=L1[:], scalar1=-MAGIC,
                                            scalar2=None, op0=AOT.add)
                    U1 = t256("xm2")
                    nc.vector.tensor_scalar(out=U1[:], in0=XY2[:], scalar1=0.5,
                                            scalar2=MAGIC, op0=AOT.add, op1=AOT.add)
                    Ut = t256("ut")
                    nc.vector.tensor_scalar(out=Ut[:], in0=U1[:], scalar1=-MAGIC,
                                            scalar2=None, op0=AOT.add)
                    D2 = t256("d2")
                    nc.vector.scalar_tensor_tensor(
                        out=D2[:], in0=XY2[:], scalar=1.0, in1=Ut[:],
                        op0=AOT.add, op1=AOT.subtract)
                    S0 = t256("xm1")
                    tt(S0[:], Ut[:], Lt[:], AOT.subtract)
                    SEL = t256("sel")
                    nc.vector.tensor_scalar(out=SEL[:], in0=S0[:], scalar1=1.0,
                                            scalar2=None, op0=AOT.min)
                    OUTSB = pl.tile([128, 32], F32, tag="outsb", name="OUTSB")
                    bst[b] = dict(D2=D2, SEL=SEL, Lt=Lt, pt16=pt16, OUTSB=OUTSB)

                # ---- stage B (per unit): weights, cell indices -------------
                for u, (b, th) in enumerate(UNITS):
                    co = 128 * th
                    D2, SEL, Lt = bst[b]["D2"], bst[b]["SEL"], bst[b]["Lt"]

                    def xsl(src, off):       # (m, l) iter over x cols
                        return _ap3(src[:], [(8, 16), (2, 4)], co + off)

                    # F [128, 64] col 16l + m : xl*128 + yl  (emitted first so
                    # the gathers can start as early as possible)
                    Ft = pl.tile([128, 64], F32, tag="ft", name="Ft")
                    nc.vector.scalar_tensor_tensor(
                        out=Ft[:], in0=_ap3(Lt[:], [(2, 4), (8, 16)], co),
                        scalar=128.0, in1=_ap3(Lt[:], [(2, 4), (8, 16)], co + 1),
                        op0=AOT.mult, op1=AOT.add)

                    # idx psum via one-hot partition-fold matmuls
                    pidx = ps.tile([128, 512], F32, tag="pcf", bufs=2, name="pidx")
                    for hh in range(8):
                        nc.tensor.matmul(
                            out=pidx[:, 64 * hh:64 * hh + 64],
                            lhsT=emat[:, 128 * hh:128 * hh + 128],
                            rhs=Ft[:], start=True, stop=True)
                    IDX = pl.tile([128, 512], I16, tag="idx", bufs=NU, name="IDX")
                    nc.vector.tensor_copy(
                        out=_ap3(IDX[:], [(1, 8), (128, 4), (8, 16)], 0),
                        in_=_ap3(pidx[:], [(64, 8), (16, 4), (1, 16)], 0))

                    AX1 = pl.tile([128, 64], F32, tag="ax1", name="AX1")
                    nc.vector.tensor_tensor(out=AX1[:], in0=xsl(D2, 0),
                                            in1=xsl(SEL, 0), op=AOT.mult)
                    AX0 = pl.tile([128, 64], F32, tag="ax0", name="AX0")
                    nc.vector.tensor_scalar(out=AX0[:], in0=AX1[:], scalar1=-1.0,
                                            scalar2=1.0, op0=AOT.mult, op1=AOT.add)
                    AY1 = pl.tile([128, 64], F32, tag="ay1", name="AY1")
                    nc.vector.tensor_tensor(out=AY1[:], in0=xsl(D2, 1),
                                            in1=xsl(SEL, 1), op=AOT.mult)
                    AY0 = pl.tile([128, 64], F32, tag="ay0", name="AY0")
                    nc.vector.tensor_scalar(out=AY0[:], in0=AY1[:], scalar1=-1.0,
                                            scalar2=1.0, op0=AOT.mult, op1=AOT.add)

                    # weights Wt [128, 256] fp16, col 16m + 4l + c
                    Wt = pl.tile([128, 256], FP16, tag="wt", bufs=NU, name="Wt")
                    for i, axt in ((0, AX0), (1, AX1)):
                        for j, ayt in ((0, AY0), (1, AY1)):
                            nc.vector.tensor_tensor(
                                out=_ap3(Wt[:], [(16, 16), (4, 4)], 2 * i + j),
                                in0=axt[:], in1=ayt[:], op=AOT.mult)
                    st[u]["IDX"] = IDX
                    st[u]["Wt"] = Wt

                # ---- stage C (per unit): gathers ---------------------------
                for u, (b, th) in enumerate(UNITS):
                    G = pl.tile([128, 8192], FP16, tag="g", bufs=3, name="G")
                    for l in range(L):
                        nc.gpsimd.dma_gather(
                            out_ap=G[:, 2048 * l:2048 * (l + 1)]
                                .rearrange("q (j e) -> q j e", e=128),
                            in_ap=ct.ap()[b * 4 + l],
                            idxs_ap=st[u]["IDX"][:, 128 * l:128 * (l + 1)],
                            num_idxs=2048, num_idxs_reg=2048,
                            elem_size=128, single_packet=False)
                    st[u]["G"] = G

                # ---- stage D (per unit): weight mult + add-tree ------------
                for u, (b, th) in enumerate(UNITS):
                    G, Wt = st[u]["G"], st[u]["Wt"]
                    GW = pl.tile([128, 8192], FP16, tag="gw", name="GW")
                    for l in range(4):
                        nc.vector.tensor_tensor(
                            out=_ap3(GW[:], [(512, 16), (16, 32), (1, 4)], 4 * l),
                            in0=_ap3(G[:], [(128, 16), (4, 32), (1, 4)], 2048 * l),
                            in1=_ap3(Wt[:], [(16, 16), (0, 32), (1, 4)], 4 * l),
                            op=AOT.mult)
                    T1 = pl.tile([128, 4096], FP16, tag="t1", name="T1")
                    nc.vector.tensor_tensor(
                        out=_ap3(T1[:], [(256, 16), (8, 32), (1, 8)], 0),
                        in0=_ap3(GW[:], [(512, 16), (16, 32), (1, 8)], 0),
                        in1=_ap3(GW[:], [(512, 16), (16, 32), (1, 8)], 8),
                        op=AOT.add)
                    T2 = pl.tile([128, 2048], FP16, tag="t2", name="T2")
                    nc.vector.tensor_tensor(
                        out=_ap3(T2[:], [(128, 16), (4, 32), (1, 4)], 0),
                        in0=_ap3(T1[:], [(256, 16), (8, 32), (1, 4)], 0),
                        in1=_ap3(T1[:], [(256, 16), (8, 32), (1, 4)], 4),
                        op=AOT.add)
                    T3 = pl.tile([128, 1024], FP16, tag="t3", name="T3")
                    nc.vector.tensor_tensor(
                        out=_ap3(T3[:], [(64, 16), (2, 32), (1, 2)], 0),
                        in0=_ap3(T2[:], [(128, 16), (4, 32), (1, 2)], 0),
                        in1=_ap3(T2[:], [(128, 16), (4, 32), (1, 2)], 2),
                        op=AOT.add)
                    CFN = pl.tile([128, 512], F32, tag="cfn", name="CFN")
                    nc.vector.tensor_tensor(
                        out=_ap3(CFN[:], [(32, 16), (1, 32)], 0),
                        in0=_ap3(T3[:], [(64, 16), (2, 32)], 0),
                        in1=_ap3(T3[:], [(64, 16), (2, 32)], 1),
                        op=AOT.add)
                    st[u]["CFN"] = CFN

                # ---- stage E (per unit): transpose cf, +net0, A0 -----------
                # u<2 copies on the scalar engine, u>=2 on DVE (engine split)
                for u, (b, th) in enumerate(UNITS):
                    CFN = st[u]["CFN"]
                    pcf = ps.tile([128, 512], F32, tag="pcf", bufs=2, name="pcf")
                    for k in range(4):
                        nc.tensor.transpose(
                            out=pcf[:, 128 * k:128 * (k + 1)],
                            in_=CFN[:, 128 * k:128 * (k + 1)],
                            identity=ident[:])
                    CFX = pl.tile([128, 512], FP16, tag="cfx", bufs=2 * NU, name="CFX")
                    nc.scalar.activation(out=CFX[:], in_=pcf[:], func=AFT.Copy)
                    # net0^T (+bias row) in its own psum bank
                    NT = ps.tile([128, 512], F32, tag="nt", bufs=1, name="NT")
                    nc.tensor.matmul(
                        out=NT[:],
                        lhsT=wrep[:],
                        rhs=_ap3(bst[b]["pt16"][:], [(1, 4), (8, 128)], 4 * th),
                        start=True, stop=True)
                    NTS = pl.tile([128, 512], FP16, tag="nts", bufs=2, name="NTS")
                    nc.scalar.activation(out=NTS[:], in_=NT[:], func=AFT.Copy)
                    A0 = pl.tile([128, 512], FP16, tag="net", bufs=2 * NU, name="A0")
                    nc.vector.tensor_tensor(out=A0[:], in0=CFX[:], in1=NTS[:],
                                            op=AOT.add)
                    st[u]["CFX"] = CFX
                    st[u]["A"] = A0

                # ---- stage F: MLP, block-interleaved across units ----------
                # all point-ops on the scalar engine: in the steady-state
                # timing loop, iteration i+1's DVE/Pool/DMA frontend overlaps
                # iteration i's ACT/PE MLP tail.
                SPLIT_U = NU

                def relu_op(u, out_ap, in_ap, bias_col=None):
                    if u < SPLIT_U:
                        nc.scalar.activation(
                            out=out_ap, in_=in_ap, func=AFT.Relu,
                            bias=(0.0 if bias_col is None else bias_col))
                    elif bias_col is None:
                        nc.vector.tensor_scalar(out=out_ap, in0=in_ap, scalar1=0.0,
                                                scalar2=None, op0=AOT.max)
                    else:
                        nc.vector.tensor_scalar(out=out_ap, in0=in_ap,
                                                scalar1=bias_col, scalar2=0.0,
                                                op0=AOT.add, op1=AOT.max)

                def bias_op(u, out_ap, in_ap, bias_col):
                    if u < SPLIT_U:
                        nc.scalar.activation(out=out_ap, in_=in_ap,
                                             func=AFT.Identity, bias=bias_col)
                    else:
                        nc.vector.tensor_scalar(out=out_ap, in0=in_ap,
                                                scalar1=bias_col, scalar2=None,
                                                op0=AOT.add)

                for blk in range(5):
                    for u in range(NU):
                        R0 = pl.tile([128, 512], FP16, tag="r0", bufs=NU, name="R0")
                        relu_op(u, R0[:], st[u]["A"][:])
                        st[u]["R0"] = R0
                    for u in range(NU):
                        ph = ps.tile([128, 512], F32, tag="ph", bufs=2, name="ph")
                        nc.tensor.matmul(out=ph[:],
                                         lhsT=wbd[:, 256 * blk:256 * blk + 128],
                                         rhs=st[u]["R0"][:], start=True, stop=True)
                        st[u]["ph"] = ph
                    for u in range(NU):
                        R1 = pl.tile([128, 512], FP16, tag="r1", bufs=NU, name="R1")
                        relu_op(u, R1[:], st[u]["ph"][:],
                                cst[:, _CN_B0 + blk:_CN_B0 + blk + 1])
                        st[u]["R1"] = R1
                    for u in range(NU):
                        pd = ps.tile([128, 512], F32, tag="pd", bufs=2, name="pd")
                        nc.tensor.matmul(out=pd[:],
                                         lhsT=wbd[:, 256 * blk + 128:256 * blk + 256],
                                         rhs=st[u]["R1"][:], start=True, stop=False)
                        last = blk == 4
                        nc.tensor.matmul(out=pd[:], lhsT=identh[:],
                                         rhs=st[u]["A"][:], start=False, stop=last)
                        if not last:
                            nc.tensor.matmul(out=pd[:], lhsT=identh[:],
                                             rhs=st[u]["CFX"][:],
                                             start=False, stop=True)
                        st[u]["pd"] = pd
                    for u in range(NU):
                        An = pl.tile([128, 512], FP16, tag="net", bufs=2 * NU,
                                     name="An")
                        bias_op(u, An[:], st[u]["pd"][:],
                                cst[:, _CN_B1 + blk:_CN_B1 + blk + 1])
                        st[u]["A"] = An

                # ---- stage G (per unit): head + output ---------------------
                for u, (b, th) in enumerate(UNITS):
                    RO = pl.tile([128, 512], FP16, tag="r0", bufs=NU, name="RO")
                    relu_op(u, RO[:], st[u]["A"][:])
                    po = ps.tile([4, 512], F32, tag="po", bufs=1, name="po")
                    nc.tensor.matmul(out=po[:], lhsT=fob[:], rhs=RO[:],
                                     start=True, stop=True)
                    OSB = pl.tile([4, 512], F32, tag="osb", name="OSB")
                    nc.scalar.activation(out=OSB[:], in_=po[:], func=AFT.Identity,
                                         bias=cst[0:4, _CN_FOB:_CN_FOB + 1])
                    pt2 = ps.tile([128, 16], F32, tag="po", bufs=1, name="pt2")
                    for kk in range(4):
                        nc.tensor.transpose(out=pt2[:, 4 * kk:4 * kk + 4],
                                            in_=OSB[:, 128 * kk:128 * kk + 128],
                                            identity=ident[0:4, 0:4])
                    nc.scalar.activation(
                        out=bst[b]["OUTSB"][:, 16 * th:16 * th + 16], in_=pt2[:],
                        func=AFT.Copy)
                    if th == 1:
                        nc.sync.dma_start(
                            out=o_d.ap()[b].rearrange("(q a) -> q a", a=32),
                            in_=bst[b]["OUTSB"][:])

            if iters:
                with tc.For_i(0, iters, 1) as _i:
                    body()
            else:
                for _ in range(unroll):
                    body()

    nc.compile()
    return nc


def _host_consts(p, c, C_mat, fc_p_W, fc_p_b, blocks_W0, blocks_b0,
                 blocks_W1, blocks_b1, fc_out_W, fc_out_b):
    """Per-core input maps (shared const tensors + per-core slices)."""
    p = np.asarray(p, np.float32)
    c = np.asarray(c, np.float32)
    C_mat = np.asarray(C_mat, np.float32)

    # supercell table: [B, L, H*W, 4*D] fp16, rows (d, corner)-interleaved
    pad = np.zeros((B, L, H + 1, W + 1, D), np.float32)
    pad[:, :, :H, :W] = c
    ctab = np.empty((B, L, H, W, 4, D), np.float32)
    ctab[..., 0, :] = pad[:, :, :H, :W]
    ctab[..., 1, :] = pad[:, :, :H, 1:W + 1]
    ctab[..., 2, :] = pad[:, :, 1:H + 1, :W]
    ctab[..., 3, :] = pad[:, :, 1:H + 1, 1:W + 1]
    ctab = np.ascontiguousarray(ctab.transpose(0, 1, 2, 3, 5, 4))  # (..., d, c)
    ctab = ctab.reshape(B, L, H * W, 4 * D).astype(np.float16)

    wbd = np.zeros((128, 1280), np.float16)
    for blk in range(5):
        for g in range(4):
            wbd[32 * g:32 * g + 32, 256 * blk + 32 * g:256 * blk + 32 * g + 32] = blocks_W0[blk]
            wbd[32 * g:32 * g + 32, 256 * blk + 128 + 32 * g:256 * blk + 160 + 32 * g] = blocks_W1[blk]

    emat = np.zeros((128, 1024), np.float32)
    for hh in range(8):
        for m in range(128):
            emat[hh * 16 + (m % 16), 128 * hh + m] = 1.0

    # wrep [16, 128]: block-diag rows 4*s2+j -> cols 32*s2+d:
    # W[j, d] for j<3, fc_p_b[d] for j==3
    wrep = np.zeros((16, 128), np.float16)
    Wp = np.asarray(fc_p_W, np.float32)
    bp = np.asarray(fc_p_b, np.float32)
    for s2 in range(4):
        for j in range(3):
            wrep[4 * s2 + j, 32 * s2:32 * s2 + 32] = Wp[j]
        wrep[4 * s2 + 3, 32 * s2:32 * s2 + 32] = bp

    fob = np.zeros((128, 4), np.float16)
    for g in range(4):
        fob[32 * g:32 * g + 32, g] = np.asarray(fc_out_W, np.float32)[:, 0]

    # pt16 [B, 16, T/4]: row 4*s2+j, col q*8+4*th+k  =  p[t, j] (1.0 for j=3)
    # where t = q*32 + 16*th + 4*k + s2 = 4*col + s2
    p4 = np.concatenate([p, np.ones((B, T, 1), np.float32)], axis=2)  # [B,T,4]
    pt16 = p4.reshape(B, T // 4, 4, 4).transpose(0, 2, 3, 1)  # [B, s2, j, t//4]
    pt16 = np.ascontiguousarray(pt16.reshape(B, 16, T // 4)).astype(np.float16)

    rint = 1.0 / np.float64(INTERVAL)
    in_maps = []
    for cc in range(NCORES):
        cst = np.zeros((128, _CN), np.float32)
        for b in range(BPC):
            gb = BPC * cc + b
            for l in range(L):
                den = np.float32(C_mat[gb, l, 3, 0] + np.float32(0.05))
                scale = rint / (np.float64(MAX_DIM) * np.float64(den))
                for ch in range(2):
                    col = b * 8 + 2 * l + ch
                    for j in range(3):
                        cst[:, _CN_C0 + 16 * j + col] = np.float32(
                            np.float64(C_mat[gb, l, ch, j]) * scale)
                    cst[:, _CN_CC + col] = np.float32(rint)
        for blk in range(5):
            cst[:, _CN_B0 + blk] = np.tile(np.asarray(blocks_b0[blk], np.float32), 4)
            cst[:, _CN_B1 + blk] = np.tile(np.asarray(blocks_b1[blk], np.float32), 4)
        cst[:, _CN_FOB] = np.float32(fc_out_b[0])
        in_maps.append({
            "ct": np.ascontiguousarray(
                ctab[BPC * cc:BPC * cc + BPC].reshape(BPC * L, H * W, 4 * D)),
            "p": np.ascontiguousarray(p[BPC * cc:BPC * cc + BPC]),
            "pt16": np.ascontiguousarray(pt16[BPC * cc:BPC * cc + BPC]),
            "cst": cst,
            "wbd": wbd,
            "emat": emat,
            "wrep": wrep,
            "fob": fob,
        })
    return in_maps


def kernel(p, z, c, C_mat, fc_p_W, fc_p_b, blocks_W0, blocks_b0,
           blocks_W1, blocks_b1, fc_out_W, fc_out_b):
    if "nc" not in _cache:
        _cache["nc"] = _build_nc()
    nc = _cache["nc"]
    in_maps = _host_consts(p, c, C_mat, fc_p_W, fc_p_b, blocks_W0, blocks_b0,
                           blocks_W1, blocks_b1, fc_out_W, fc_out_b)
    res = run_bass_kernel_spmd(nc, in_maps, core_ids=list(range(NCORES)))
    out = np.empty((B, T), np.float32)
    for cc in range(NCORES):
        out[BPC * cc:BPC * cc + BPC] = res.results[cc]["o"]
    return out


# revision 35
# speedup vs baseline: 3.7757x; 3.7757x over previous
"""Trainium2 Bass kernel for nn_DecoderCBatchNorm_63788854280467.

Decoder with bilinear plane-feature interpolation + small residual MLP.
Data-parallel over batch: 16 batches -> 8 NeuronCores (2 each).

v2 layout strategy:
- Supercell table rows are (d, corner)-interleaved fp16, so the gather lands
  features in a layout where the bilinear weight multiply is a fully packed
  16-bit DVE op (2x rate) and the 16 (view, corner) terms per (point, d) are
  contiguous for a packed fp16 add-tree.
- Projection/clamp chain uses host-folded coefficients (C/(0.55*den*interval))
  so xy comes from 6 wide vector ops instead of exact-division ladders.
- MLP matmuls, residual adds (via identity-matmul PSUM accumulation), and
  transposes all run 16-bit on the PE; relu/copies ride the scalar engine.
"""

import sys

sys.path.insert(0, "/opt/trn_rl_repo")

import numpy as np

import concourse.bass as bass
import concourse.bacc as bacc
import concourse.mybir as mybir
from concourse import tile, library_config
from concourse.bass_utils import run_bass_kernel_spmd
from concourse.masks import make_identity

F32 = mybir.dt.float32
FP16 = mybir.dt.float16
I16 = mybir.dt.int16
AOT = mybir.AluOpType
AFT = mybir.ActivationFunctionType

B, T, L, H, W, D = 16, 4096, 4, 128, 128, 32
MAX_DIM = 0.55
NCORES = 8
BPC = B // NCORES          # batches per core = 2
MAGIC = 12582912.0         # 1.5 * 2^23 : f32 RNE rounding constant
INTERVAL = np.float32(2.0) / np.float32(H - 1)   # f32(2/127), matches jnp

# cst column map (per-batch blocks of 8 = (l, coord) pairs)
_CN_C0 = 0        # 16 cols: coef for p0, idx b*8 + 2l+c
_CN_C1 = 16
_CN_C2 = 32
_CN_CC = 48       # 16 cols: additive const (= 1/interval)
_CN_B0 = 64       # 5 cols
_CN_B1 = 69       # 5 cols
_CN_FOB = 74      # 1 col
_CN = 75

_cache = {}


def _ap3(tile_ap, dims, offset_elems):
    """Build an AP with explicit free dims [(step, count), ...] on a tile AP."""
    base = tile_ap
    ap = [list(base.ap[0])] + [[s, c] for (s, c) in dims]
    return bass.AP(base.tensor, base.offset + offset_elems, ap)


def _build_nc(iters=0, unroll=1, ablate=()):
    """Build the per-core program. iters>0 wraps the body in a timing loop.

    ablate: subset of {"gather", "phasec", "mlp", "xy"} — replaces that stage
    with cheap stand-ins (wrong results; timing-bisect only).
    """
    ablate = frozenset(ablate)
    nc = bacc.Bacc("TRN2", target_bir_lowering=False, debug=False)

    ct = nc.dram_tensor("ct", [BPC * L, H * W, 4 * D], FP16, kind="ExternalInput")
    p_d = nc.dram_tensor("p", [BPC, T, 3], F32, kind="ExternalInput")
    pt16_d = nc.dram_tensor("pt16", [BPC, 16, T // 4], FP16, kind="ExternalInput")
    cst_d = nc.dram_tensor("cst", [128, _CN], F32, kind="ExternalInput")
    wbd_d = nc.dram_tensor("wbd", [128, 1280], FP16, kind="ExternalInput")
    emat_d = nc.dram_tensor("emat", [128, 1024], F32, kind="ExternalInput")
    wrep_d = nc.dram_tensor("wrep", [16, 128], FP16, kind="ExternalInput")
    fob_d = nc.dram_tensor("fob", [128, 4], FP16, kind="ExternalInput")
    o_d = nc.dram_tensor("o", [BPC, T], F32, kind="ExternalOutput")

    with tile.TileContext(nc) as tc:
        nc.gpsimd.load_library(library_config.mlp)
        with tc.tile_pool(name="sb", bufs=2) as pl, \
             tc.tile_pool(name="cs", bufs=1) as cs, \
             tc.tile_pool(name="ps", bufs=1, space="PSUM") as ps:

            ident = cs.tile([128, 128], F32)
            make_identity(nc, ident[:])
            identh = cs.tile([128, 128], FP16)
            nc.vector.tensor_copy(out=identh[:], in_=ident[:])
            cst = cs.tile([128, _CN], F32)
            nc.sync.dma_start(out=cst[:], in_=cst_d.ap())
            wbd = cs.tile([128, 1280], FP16)
            nc.sync.dma_start(out=wbd[:], in_=wbd_d.ap())
            emat = cs.tile([128, 1024], F32)
            nc.sync.dma_start(out=emat[:], in_=emat_d.ap())
            wrep = cs.tile([16, 128], FP16)
            nc.sync.dma_start(out=wrep[:], in_=wrep_d.ap())
            fob = cs.tile([128, 4], FP16)
            nc.sync.dma_start(out=fob[:], in_=fob_d.ap())
            gshared = cfnshared = None
            if "gather" in ablate:
                gshared = cs.tile([128, 8192], FP16)
                nc.vector.memset(gshared[:], 0.25)
            if "phasec" in ablate:
                cfnshared = cs.tile([128, 512], F32)
                nc.vector.memset(cfnshared[:], 0.25)

            NU = 2 * BPC      # pipeline units: (b, th)
            UNITS = [(b, th) for b in range(BPC) for th in range(2)]

            def body():
                st = {u: {} for u in range(NU)}   # per-unit tiles
                bst = {}                          # per-batch tiles

                def tt(o, a, bb, op):
                    nc.vector.tensor_tensor(out=o, in0=a, in1=bb, op=op)

                # ---- stage A (per batch): loads + xy/round chain -----------
                for b in range(BPC):
                    p_sb = pl.tile([128, 96], F32, tag="p", name="p_sb")
                    nc.sync.dma_start(
                        out=p_sb[:],
                        in_=p_d.ap()[b].rearrange("(q a) j -> q (a j)", a=32))
                    pt16 = pl.tile([16, T // 4], FP16, tag="pt16", name="pt16")
                    nc.sync.dma_start(out=pt16[:], in_=pt16_d.ap()[b])

                    def cc(base):   # per-column const broadcast over m
                        return _ap3(cst[:], [(0, 32), (1, 8)], base + 8 * b)

                    def pj(j):      # p coord j broadcast over the 8 lc cols
                        return _ap3(p_sb[:], [(3, 32), (0, 8)], j)

                    def t256(tag):
                        return pl.tile([128, 256], F32, tag=tag, name=tag)

                    # xy = sum_j p_j * coef_j + const  [128, 8m+lc]
                    M1t = t256("xm1")
                    tt(M1t[:], pj(0), cc(_CN_C0), AOT.mult)
                    M2t = t256("xm2")
                    tt(M2t[:], pj(1), cc(_CN_C1), AOT.mult)
                    M12 = t256("xm12")
                    tt(M12[:], M1t[:], M2t[:], AOT.add)
                    M3t = t256("xm1")
                    tt(M3t[:], pj(2), cc(_CN_C2), AOT.mult)
                    M3c = t256("xm2")
                    tt(M3c[:], M3t[:], cc(_CN_CC), AOT.add)
                    XYC = t256("xm1")
                    tt(XYC[:], M12[:], M3c[:], AOT.add)

                    # clamp + where(>=127 -> 126.9)
                    XY1 = t256("xm2")
                    nc.vector.tensor_scalar(out=XY1[:], in0=XYC[:], scalar1=0.0,
                                            scalar2=200.0, op0=AOT.max, op1=AOT.min)
                    Msk = t256("xm12")
                    nc.vector.tensor_scalar(out=Msk[:], in0=XY1[:], scalar1=127.0,
                                            scalar2=None, op0=AOT.is_ge)
                    Dd = t256("xm1")
                    nc.vector.tensor_scalar(out=Dd[:], in0=XY1[:], scalar1=126.9,
                                            scalar2=None, op0=AOT.subtract)
                    MD = t256("xm3")
                    tt(MD[:], Msk[:], Dd[:], AOT.mult)
                    XY2 = t256("xy2")
                    tt(XY2[:], XY1[:], MD[:], AOT.subtract)

                    # round L/U (RNE), D2 = 1-dx, SEL
                    L1 = t256("xm1")
                    nc.vector.tensor_scalar(out=L1[:], in0=XY2[:], scalar1=-0.5,
                                            scalar2=MAGIC, op0=AOT.add, op1=AOT.add)
                    Lt = t256("lt")
                    nc.vector.tensor_scalar(out=Lt[:], in0=L1[:], scalar1=-MAGIC,
                                            scalar2=None, op0=AOT.add)
                    U1 = t256("xm2")
                    nc.vector.tensor_scalar(out=U1[:], in0=XY2[:], scalar1=0.5,
                                            scalar2=MAGIC, op0=AOT.add, op1=AOT.add)
                    Ut = t256("ut")
                    nc.vector.tensor_scalar(out=Ut[:], in0=U1[:], scalar1=-MAGIC,
                                            scalar2=None, op0=AOT.add)
                    D2 = t256("d2")
                    nc.vector.scalar_tensor_tensor(
                        out=D2[:], in0=XY2[:], scalar=1.0, in1=Ut[:],
                        op0=AOT.add, op1=AOT.subtract)
                    S0 = t256("xm1")
                    tt(S0[:], Ut[:], Lt[:], AOT.subtract)
                    SEL = t256("sel")
                    nc.vector.tensor_scalar(out=SEL[:], in0=S0[:], scalar1=1.0,
                                            scalar2=None, op0=AOT.min)
                    OUTSB = pl.tile([128, 32], F32, tag="outsb", name="OUTSB")
                    bst[b] = dict(D2=D2, SEL=SEL, Lt=Lt, pt16=pt16, OUTSB=OUTSB)

                # ---- stage B (per unit): weights, cell indices -------------
                for u, (b, th) in enumerate(UNITS):
                    co = 128 * th
                    D2, SEL, Lt = bst[b]["D2"], bst[b]["SEL"], bst[b]["Lt"]

                    def xsl(src, off):       # (m, l) iter over x cols
                        return _ap3(src[:], [(8, 16), (2, 4)], co + off)

                    # F [128, 64] col 16l + m : xl*128 + yl  (emitted first so
                    # the gathers can start as early as possible)
                    Ft = pl.tile([128, 64], F32, tag="ft", name="Ft")
                    nc.vector.scalar_tensor_tensor(
                        out=Ft[:], in0=_ap3(Lt[:], [(2, 4), (8, 16)], co),
                        scalar=128.0, in1=_ap3(Lt[:], [(2, 4), (8, 16)], co + 1),
                        op0=AOT.mult, op1=AOT.add)

                    # idx psum via one-hot partition-fold matmuls
                    pidx = ps.tile([128, 512], F32, tag="pcf", bufs=2, name="pidx")
                    for hh in range(8):
                        nc.tensor.matmul(
                            out=pidx[:, 64 * hh:64 * hh + 64],
                            lhsT=emat[:, 128 * hh:128 * hh + 128],
                            rhs=Ft[:], start=True, stop=True)
                    IDX = pl.tile([128, 512], I16, tag="idx", bufs=NU, name="IDX")
                    nc.vector.tensor_copy(
                        out=_ap3(IDX[:], [(1, 8), (128, 4), (8, 16)], 0),
                        in_=_ap3(pidx[:], [(64, 8), (16, 4), (1, 16)], 0))

                    AX1 = pl.tile([128, 64], F32, tag="ax1", name="AX1")
                    nc.vector.tensor_tensor(out=AX1[:], in0=xsl(D2, 0),
                                            in1=xsl(SEL, 0), op=AOT.mult)
                    AX0 = pl.tile([128, 64], F32, tag="ax0", name="AX0")
                    nc.vector.tensor_scalar(out=AX0[:], in0=AX1[:], scalar1=-1.0,
                                            scalar2=1.0, op0=AOT.mult, op1=AOT.add)
                    AY1 = pl.tile([128, 64], F32, tag="ay1", name="AY1")
                    nc.vector.tensor_tensor(out=AY1[:], in0=xsl(D2, 1),
                                            in1=xsl(SEL, 1), op=AOT.mult)
                    AY0 = pl.tile([128, 64], F32, tag="ay0", name="AY0")
                    nc.vector.tensor_scalar(out=AY0[:], in0=AY1[:], scalar1=-1.0,
                                            scalar2=1.0, op0=AOT.mult, op1=AOT.add)

                    # weights Wt [128, 256] fp16, col 16m + 4l + c
                    Wt = pl.tile([128, 256], FP16, tag="wt", bufs=NU, name="Wt")
                    for i, axt in ((0, AX0), (1, AX1)):
                        for j, ayt in ((0, AY0), (1, AY1)):
                            nc.vector.tensor_tensor(
                                out=_ap3(Wt[:], [(16, 16), (4, 4)], 2 * i + j),
                                in0=axt[:], in1=ayt[:], op=AOT.mult)
                    st[u]["IDX"] = IDX
                    st[u]["Wt"] = Wt

                # ---- stage C (per unit): gathers ---------------------------
                for u, (b, th) in enumerate(UNITS):
                    if "gather" in ablate:
                        st[u]["G"] = gshared
                        continue
                    G = pl.tile([128, 8192], FP16, tag="g", bufs=3, name="G")
                    for l in range(L):
                        nc.gpsimd.dma_gather(
                            out_ap=G[:, 2048 * l:2048 * (l + 1)]
                                .rearrange("q (j e) -> q j e", e=128),
                            in_ap=ct.ap()[b * 4 + l],
                            idxs_ap=st[u]["IDX"][:, 128 * l:128 * (l + 1)],
                            num_idxs=2048, num_idxs_reg=2048,
                            elem_size=128, single_packet=False)
                    st[u]["G"] = G

                # ---- stage D (per unit): weight mult + add-tree ------------
                for u, (b, th) in enumerate(UNITS):
                    if "phasec" in ablate:
                        st[u]["CFN"] = cfnshared
                        continue
                    G, Wt = st[u]["G"], st[u]["Wt"]
                    GW = pl.tile([128, 8192], FP16, tag="gw", name="GW")
                    for l in range(4):
                        nc.vector.tensor_tensor(
                            out=_ap3(GW[:], [(512, 16), (16, 32), (1, 4)], 4 * l),
                            in0=_ap3(G[:], [(128, 16), (4, 32), (1, 4)], 2048 * l),
                            in1=_ap3(Wt[:], [(16, 16), (0, 32), (1, 4)], 4 * l),
                            op=AOT.mult)
                    T1 = pl.tile([128, 4096], FP16, tag="t1", name="T1")
                    nc.vector.tensor_tensor(
                        out=_ap3(T1[:], [(256, 16), (8, 32), (1, 8)], 0),
                        in0=_ap3(GW[:], [(512, 16), (16, 32), (1, 8)], 0),
                        in1=_ap3(GW[:], [(512, 16), (16, 32), (1, 8)], 8),
                        op=AOT.add)
                    T2 = pl.tile([128, 2048], FP16, tag="t2", name="T2")
                    nc.vector.tensor_tensor(
                        out=_ap3(T2[:], [(128, 16), (4, 32), (1, 4)], 0),
                        in0=_ap3(T1[:], [(256, 16), (8, 32), (1, 4)], 0),
                        in1=_ap3(T1[:], [(256, 16), (8, 32), (1, 4)], 4),
                        op=AOT.add)
                    T3 = pl.tile([128, 1024], FP16, tag="t3", name="T3")
                    nc.vector.tensor_tensor(
                        out=_ap3(T3[:], [(64, 16), (2, 32), (1, 2)], 0),
                        in0=_ap3(T2[:], [(128, 16), (4, 32), (1, 2)], 0),
                        in1=_ap3(T2[:], [(128, 16), (4, 32), (1, 2)], 2),
                        op=AOT.add)
                    CFN = pl.tile([128, 512], F32, tag="cfn", name="CFN")
                    nc.vector.tensor_tensor(
                        out=_ap3(CFN[:], [(32, 16), (1, 32)], 0),
                        in0=_ap3(T3[:], [(64, 16), (2, 32)], 0),
                        in1=_ap3(T3[:], [(64, 16), (2, 32)], 1),
                        op=AOT.add)
                    st[u]["CFN"] = CFN

                # ---- stage E (per unit): transpose cf, +net0, A0 -----------
                # u<2 copies on the scalar engine, u>=2 on DVE (engine split)
                for u, (b, th) in enumerate(UNITS):
                    CFN = st[u]["CFN"]
                    pcf = ps.tile([128, 512], F32, tag="pcf", bufs=2, name="pcf")
                    for k in range(4):
                        nc.tensor.transpose(
                            out=pcf[:, 128 * k:128 * (k + 1)],
                            in_=CFN[:, 128 * k:128 * (k + 1)],
                            identity=ident[:])
                    CFX = pl.tile([128, 512], FP16, tag="cfx", bufs=2 * NU, name="CFX")
                    nc.scalar.activation(out=CFX[:], in_=pcf[:], func=AFT.Copy)
                    # net0^T (+bias row) in its own psum bank
                    NT = ps.tile([128, 512], F32, tag="nt", bufs=1, name="NT")
                    nc.tensor.matmul(
                        out=NT[:],
                        lhsT=wrep[:],
                        rhs=_ap3(bst[b]["pt16"][:], [(1, 4), (8, 128)], 4 * th),
                        start=True, stop=True)
                    NTS = pl.tile([128, 512], FP16, tag="nts", bufs=2, name="NTS")
                    nc.scalar.activation(out=NTS[:], in_=NT[:], func=AFT.Copy)
                    A0 = pl.tile([128, 512], FP16, tag="net", bufs=2 * NU, name="A0")
                    nc.vector.tensor_tensor(out=A0[:], in0=CFX[:], in1=NTS[:],
                                            op=AOT.add)
                    st[u]["CFX"] = CFX
                    st[u]["A"] = A0

                # ---- stage F: MLP, block-interleaved across units ----------
                # all point-ops on the scalar engine: in the steady-state
                # timing loop, iteration i+1's DVE/Pool/DMA frontend overlaps
                # iteration i's ACT/PE MLP tail.
                SPLIT_U = NU

                def relu_op(u, out_ap, in_ap, bias_col=None):
                    if u < SPLIT_U:
                        nc.scalar.activation(
                            out=out_ap, in_=in_ap, func=AFT.Relu,
                            bias=(0.0 if bias_col is None else bias_col))
                    elif bias_col is None:
                        nc.vector.tensor_scalar(out=out_ap, in0=in_ap, scalar1=0.0,
                                                scalar2=None, op0=AOT.max)
                    else:
                        nc.vector.tensor_scalar(out=out_ap, in0=in_ap,
                                                scalar1=bias_col, scalar2=0.0,
                                                op0=AOT.add, op1=AOT.max)

                def bias_op(u, out_ap, in_ap, bias_col):
                    if u < SPLIT_U:
                        nc.scalar.activation(out=out_ap, in_=in_ap,
                                             func=AFT.Identity, bias=bias_col)
                    else:
                        nc.vector.tensor_scalar(out=out_ap, in0=in_ap,
                                                scalar1=bias_col, scalar2=None,
                                                op0=AOT.add)

                for blk in range(0 if "mlp" in ablate else 5):
                    for u in range(NU):
                        R0 = pl.tile([128, 512], FP16, tag="r0", bufs=NU, name="R0")
                        relu_op(u, R0[:], st[u]["A"][:])
                        st[u]["R0"] = R0
                    for u in range(NU):
                        ph = ps.tile([128, 512], F32, tag="ph", bufs=2, name="ph")
                        nc.tensor.matmul(out=ph[:],
                                         lhsT=wbd[:, 256 * blk:256 * blk + 128],
                                         rhs=st[u]["R0"][:], start=True, stop=True)
                        st[u]["ph"] = ph
                    for u in range(NU):
                        R1 = pl.tile([128, 512], FP16, tag="r1", bufs=NU, name="R1")
                        relu_op(u, R1[:], st[u]["ph"][:],
                                cst[:, _CN_B0 + blk:_CN_B0 + blk + 1])
                        st[u]["R1"] = R1
                    for u in range(NU):
                        pd = ps.tile([128, 512], F32, tag="pd", bufs=2, name="pd")
                        nc.tensor.matmul(out=pd[:],
                                         lhsT=wbd[:, 256 * blk + 128:256 * blk + 256],
                                         rhs=st[u]["R1"][:], start=True, stop=False)
                        last = blk == 4
                        nc.tensor.matmul(out=pd[:], lhsT=identh[:],
                                         rhs=st[u]["A"][:], start=False, stop=last)
                        if not last:
                            nc.tensor.matmul(out=pd[:], lhsT=identh[:],
                                             rhs=st[u]["CFX"][:],
                                             start=False, stop=True)
                        st[u]["pd"] = pd
                    for u in range(NU):
                        An = pl.tile([128, 512], FP16, tag="net", bufs=2 * NU,
                                     name="An")
                        bias_op(u, An[:], st[u]["pd"][:],
                                cst[:, _CN_B1 + blk:_CN_B1 + blk + 1])
                        st[u]["A"] = An

                # ---- stage G (per unit): head + output ---------------------
                for u, (b, th) in enumerate(UNITS):
                    RO = pl.tile([128, 512], FP16, tag="r0", bufs=NU, name="RO")
                    relu_op(u, RO[:], st[u]["A"][:])
                    po = ps.tile([4, 512], F32, tag="po", bufs=1, name="po")
                    nc.tensor.matmul(out=po[:], lhsT=fob[:], rhs=RO[:],
                                     start=True, stop=True)
                    OSB = pl.tile([4, 512], F32, tag="osb", name="OSB")
                    nc.scalar.activation(out=OSB[:], in_=po[:], func=AFT.Identity,
                                         bias=cst[0:4, _CN_FOB:_CN_FOB + 1])
                    pt2 = ps.tile([128, 16], F32, tag="po", bufs=1, name="pt2")
                    for kk in range(4):
                        nc.tensor.transpose(out=pt2[:, 4 * kk:4 * kk + 4],
                                            in_=OSB[:, 128 * kk:128 * kk + 128],
                                            identity=ident[0:4, 0:4])
                    nc.scalar.activation(
                        out=bst[b]["OUTSB"][:, 16 * th:16 * th + 16], in_=pt2[:],
                        func=AFT.Copy)
                    if th == 1:
                        nc.sync.dma_start(
                            out=o_d.ap()[b].rearrange("(q a) -> q a", a=32),
                            in_=bst[b]["OUTSB"][:])

            if iters:
                with tc.For_i(0, iters, 1) as _i:
                    body()
            else:
                for _ in range(unroll):
                    body()

    nc.compile()
    return nc


def _host_consts(p, c, C_mat, fc_p_W, fc_p_b, blocks_W0, blocks_b0,
                 blocks_W1, blocks_b1, fc_out_W, fc_out_b):
    """Per-core input maps (shared const tensors + per-core slices)."""
    p = np.asarray(p, np.float32)
    c = np.asarray(c, np.float32)
    C_mat = np.asarray(C_mat, np.float32)

    # supercell table: [B, L, H*W, 4*D] fp16, rows (d, corner)-interleaved
    pad = np.zeros((B, L, H + 1, W + 1, D), np.float32)
    pad[:, :, :H, :W] = c
    ctab = np.empty((B, L, H, W, 4, D), np.float32)
    ctab[..., 0, :] = pad[:, :, :H, :W]
    ctab[..., 1, :] = pad[:, :, :H, 1:W + 1]
    ctab[..., 2, :] = pad[:, :, 1:H + 1, :W]
    ctab[..., 3, :] = pad[:, :, 1:H + 1, 1:W + 1]
    ctab = np.ascontiguousarray(ctab.transpose(0, 1, 2, 3, 5, 4))  # (..., d, c)
    ctab = ctab.reshape(B, L, H * W, 4 * D).astype(np.float16)

    wbd = np.zeros((128, 1280), np.float16)
    for blk in range(5):
        for g in range(4):
            wbd[32 * g:32 * g + 32, 256 * blk + 32 * g:256 * blk + 32 * g + 32] = blocks_W0[blk]
            wbd[32 * g:32 * g + 32, 256 * blk + 128 + 32 * g:256 * blk + 160 + 32 * g] = blocks_W1[blk]

    emat = np.zeros((128, 1024), np.float32)
    for hh in range(8):
        for m in range(128):
            emat[hh * 16 + (m % 16), 128 * hh + m] = 1.0

    # wrep [16, 128]: block-diag rows 4*s2+j -> cols 32*s2+d:
    # W[j, d] for j<3, fc_p_b[d] for j==3
    wrep = np.zeros((16, 128), np.float16)
    Wp = np.asarray(fc_p_W, np.float32)
    bp = np.asarray(fc_p_b, np.float32)
    for s2 in range(4):
        for j in range(3):
            wrep[4 * s2 + j, 32 * s2:32 * s2 + 32] = Wp[j]
        wrep[4 * s2 + 3, 32 * s2:32 * s2 + 32] = bp

    fob = np.zeros((128, 4), np.float16)
    for g in range(4):
        fob[32 * g:32 * g + 32, g] = np.asarray(fc_out_W, np.float32)[:, 0]

    # pt16 [B, 16, T/4]: row 4*s2+j, col q*8+4*th+k  =  p[t, j] (1.0 for j=3)
    # where t = q*32 + 16*th + 4*k + s2 = 4*col + s2
    p4 = np.concatenate([p, np.ones((B, T, 1), np.float32)], axis=2)  # [B,T,4]
    pt16 = p4.reshape(B, T // 4, 4, 4).transpose(0, 2, 3, 1)  # [B, s2, j, t//4]
    pt16 = np.ascontiguousarray(pt16.reshape(B, 16, T // 4)).astype(np.float16)

    rint = 1.0 / np.float64(INTERVAL)
    in_maps = []
    for cc in range(NCORES):
        cst = np.zeros((128, _CN), np.float32)
        for b in range(BPC):
            gb = BPC * cc + b
            for l in range(L):
                den = np.float32(C_mat[gb, l, 3, 0] + np.float32(0.05))
                scale = rint / (np.float64(MAX_DIM) * np.float64(den))
                for ch in range(2):
                    col = b * 8 + 2 * l + ch
                    for j in range(3):
                        cst[:, _CN_C0 + 16 * j + col] = np.float32(
                            np.float64(C_mat[gb, l, ch, j]) * scale)
                    cst[:, _CN_CC + col] = np.float32(rint)
        for blk in range(5):
            cst[:, _CN_B0 + blk] = np.tile(np.asarray(blocks_b0[blk], np.float32), 4)
            cst[:, _CN_B1 + blk] = np.tile(np.asarray(blocks_b1[blk], np.float32), 4)
        cst[:, _CN_FOB] = np.float32(fc_out_b[0])
        in_maps.append({
            "ct": np.ascontiguousarray(
                ctab[BPC * cc:BPC * cc + BPC].reshape(BPC * L, H * W, 4 * D)),
            "p": np.ascontiguousarray(p[BPC * cc:BPC * cc + BPC]),
            "pt16": np.ascontiguousarray(pt16[BPC * cc:BPC * cc + BPC]),
            "cst": cst,
            "wbd": wbd,
            "emat": emat,
            "wrep": wrep,
            "fob": fob,
        })
    return in_maps


def kernel(p, z, c, C_mat, fc_p_W, fc_p_b, blocks_W0, blocks_b0,
           blocks_W1, blocks_b1, fc_out_W, fc_out_b):
    if "nc" not in _cache:
        _cache["nc"] = _build_nc()
    nc = _cache["nc"]
    in_maps = _host_consts(p, c, C_mat, fc_p_W, fc_p_b, blocks_W0, blocks_b0,
                           blocks_W1, blocks_b1, fc_out_W, fc_out_b)
    res = run_bass_kernel_spmd(nc, in_maps, core_ids=list(range(NCORES)))
    out = np.empty((B, T), np.float32)
    for cc in range(NCORES):
        out[BPC * cc:BPC * cc + BPC] = res.results[cc]["o"]
    return out
